# revision 2
# baseline (speedup 1.0000x reference)
"""3-layer GAT on 8 Trainium2 NeuronCores — v3.

Changes vs v2 baseline (1641us -> 1418us):
- Resharded dst ownership (tiered): core c owns the next layer's dst rows
  locally, so L2/L3 phase A reads its own h rows directly — the h1a (16.8MB)
  and h2a (4.2MB) AllGathers are eliminated, along with the indirect phase-A
  row gathers (plain contiguous loads now).
- Gather tables merged into half-tables written by 2 piece AllGathers each
  (fewer, larger collectives); dma_gather windows are 32768-row slices
  (int16 idx limit). L2 uses 2 buckets/chunk (16384-row halves), L3 one.
- mxc message multiply split per head with f16 exp operand.
- Edge gathers stay on dma_gather/SWDGE (measured 1.96us + 7.34ns/idx):
  indirect_dma_start drains at only ~10GB/s through the single dynamic-queue
  DMA engine (79) and congests dma_gather's drains; single_packet=True and
  multi-queue (qPoolDynamic1-3) both break on HW. See memory notes.
"""
import os
import sys

for _p in ("/opt/trn_rl_repo", "/root/.axon_site/_ro/trn_rl_repo"):
    if os.path.isdir(_p) and _p not in sys.path:
        sys.path.insert(0, _p)

import numpy as np

N0, N1, N2, N3 = 131072, 32768, 8192, 2048
H, C_HID, C_OUT, F_IN = 4, 64, 47, 100
NC_ = 8
P = 128

W1T = 128          # xt cols: [x(100) | 1 | a_s(4) | 0*23]
W2T = 384          # p1 cols: [(xl_h(64)|1)*4 =260 | a_s(4) | pad]
W3T = 256          # p2 cols: [(xl_h(47)|1)*4 =192 | a_s(4) | pad]

# tiered dst ownership: (global_start, width_per_core); local offsets cumulative
TIERS1 = [(0, 256), (2048, 768), (8192, 3072)]   # N1 -> 4096/core
TIERS2 = [(0, 256), (2048, 768)]                 # N2 -> 1024/core
TIERS3 = [(0, 256)]                              # N3 -> 256/core


def _dst_map(tiers, n):
    owner = np.zeros(n, np.int64)
    loc = np.zeros(n, np.int64)
    base = 0
    for s, w in tiers:
        g = np.arange(s, s + 8 * w)
        owner[g] = (g - s) // w
        loc[g] = base + (g - s) % w
        base += w
    return owner, loc


OWN1, LOC1 = _dst_map(TIERS1, N1)
OWN2, LOC2 = _dst_map(TIERS2, N2)
OWN3, LOC3 = _dst_map(TIERS3, N3)


def _t1_bucket_row(src):
    """x table: half k = shard half; rih = row in 65536-row half table;
    bucket = k*2 + rih//32768 (int16 gather windows); riw = rih%32768."""
    k = (src % 16384) // 8192
    rih = (src // 16384) * 8192 + (src % 8192)
    b = k * 2 + rih // 32768
    riw = rih % 32768
    return b, riw, rih


def _t2_bucket_row(src):
    """p1 table: half = local-row half (2048/core); 16384-row halves fit int16."""
    o = OWN1[src]
    p = LOC1[src]
    k = p // 2048
    rih = o * 2048 + (p % 2048)
    return k, rih, rih


def _t3_bucket_row(src):
    """p2 table: single 8192-row table."""
    o = OWN2[src]
    p = LOC2[src]
    row = o * 1024 + p
    return np.zeros_like(row), row, row


def _build_sched(src, dst, owner, loc, bucket_row_fn, n_loc, nbk):
    """Per-core edge schedule: chunks of 128 local dst, nbk buckets per chunk.

    Returns per-core dict(off [P,T_tot] i32, dloc [P,T_tot] f16,
    stT [P, T_tot*P] f16), tiles [ch][bk], n_chunks.
    """
    src = np.asarray(src, np.int64)
    dst = np.asarray(dst, np.int64)
    bk_all, riw_all, rih_all = bucket_row_fn(src)
    n_chunks = n_loc // P
    percore = []
    for c in range(NC_):
        m = owner[dst] == c
        bk = bk_all[m]
        rw = riw_all[m]
        rh = rih_all[m]
        dl = loc[dst[m]]
        ch = dl // P
        order = np.lexsort((bk, ch))
        bk, rw, rh, dl, ch = bk[order], rw[order], rh[order], dl[order], ch[order]
        counts = np.zeros((n_chunks, nbk), np.int64)
        np.add.at(counts, (ch, bk), 1)
        percore.append((rw, rh, dl, counts))
    counts_all = np.stack([p[3] for p in percore])
    ncall = counts_all.max(axis=0)
    tiles = -(-ncall // P)
    T_tot = int(tiles.sum())
    scheds = []
    for c in range(NC_):
        rw, rh, dl, counts = percore[c]
        idx16 = np.zeros(T_tot * P, np.int16)
        off32 = np.zeros(T_tot * P, np.int32)
        dloc = np.full(T_tot * P, -1.0, np.float16)
        s = 0
        epos = 0
        for i in range(n_chunks):
            for b in range(nbk):
                ne = int(counts[i, b])
                sl = slice(s * P, s * P + ne)
                idx16[sl] = rw[epos:epos + ne].astype(np.int16)
                off32[sl] = rh[epos:epos + ne]
                dloc[sl] = (dl[epos:epos + ne] - i * P).astype(np.float16)
                epos += ne
                s += int(tiles[i, b])
        assert epos == len(rw) and s == T_tot
        stT = np.zeros((P, T_tot * P), np.float16)
        real = np.nonzero(dloc >= 0)[0]
        d_ = dloc[real].astype(np.int64)
        stT[d_, real] = 1.0
        idxw = np.tile(idx16.reshape(T_tot * 8, 16).T, (8, 1))
        scheds.append(dict(
            idxw=np.ascontiguousarray(idxw),
            off=np.ascontiguousarray(off32.reshape(T_tot, P).T),
            dloc=np.ascontiguousarray(dloc.reshape(T_tot, P).T),
            stT=np.ascontiguousarray(stT)))
    return scheds, tiles.tolist(), ncall.tolist(), n_chunks


def _fold_a(W, a):
    h, c = a.shape
    out = np.zeros((W.shape[0], h), np.float32)
    for hh in range(h):
        out[:, hh] = W[:, hh * c:(hh + 1) * c] @ a[hh]
    return out


def _prep_weights(inputs):
    f16 = np.float16
    w = {}
    was1 = np.zeros((P, H), np.float32)
    was1[:F_IN] = _fold_a(inputs["W1"], inputs["as1"])
    w["was1"] = was1.astype(f16)
    wbig1 = np.zeros((P, 4 + 256), np.float32)
    wbig1[:F_IN, :4] = _fold_a(inputs["W1"], inputs["ad1"])
    wbig1[:F_IN, 4:] = inputs["Ws1"]
    wbig1[F_IN, 4:] = inputs["b1"] + inputs["bs1"]
    w["wbig1"] = wbig1.astype(f16)
    wp1 = np.zeros((P, 256), np.float32)
    wp1[:F_IN] = inputs["W1"]
    w["wp1"] = wp1.astype(f16)
    wp2 = np.zeros((256, 264), np.float32)
    for hh in range(H):
        wp2[:, hh * 65:hh * 65 + 64] = inputs["W2"][:, hh * 64:(hh + 1) * 64]
    wp2[:, 260:264] = _fold_a(inputs["W2"], inputs["as2"])
    w["wp2"] = wp2.astype(f16)
    brow2 = np.zeros((1, 264), np.float32)
    brow2[0, [64, 129, 194, 259]] = 1.0
    w["brow2"] = brow2.astype(f16)
    wbig2 = np.zeros((256, 260), np.float32)
    wbig2[:, :4] = _fold_a(inputs["W2"], inputs["ad2"])
    wbig2[:, 4:] = inputs["Ws2"]
    w["wbig2"] = wbig2.astype(f16)
    brow2b = np.zeros((1, 260), np.float32)
    brow2b[0, 4:] = inputs["b2"] + inputs["bs2"]
    w["brow2b"] = brow2b.astype(f16)
    wp3 = np.zeros((256, 196), np.float32)
    for hh in range(H):
        wp3[:, hh * 48:hh * 48 + 47] = inputs["W3"][:, hh * 47:(hh + 1) * 47]
    wp3[:, 192:196] = _fold_a(inputs["W3"], inputs["as3"])
    w["wp3"] = wp3.astype(f16)
    brow3 = np.zeros((1, 196), np.float32)
    brow3[0, [47, 95, 143, 191]] = 1.0
    w["brow3"] = brow3.astype(f16)
    wbig3 = np.zeros((256, 51), np.float32)
    wbig3[:, :4] = _fold_a(inputs["W3"], inputs["ad3"])
    wbig3[:, 4:] = inputs["Ws3"]
    w["wbig3"] = wbig3.astype(f16)
    brow3b = np.zeros((1, 51), np.float32)
    brow3b[0, 4:] = inputs["b3"] + inputs["bs3"]
    w["brow3b"] = brow3b.astype(f16)
    return w


# ---------------------------------------------------------------- bass build

def _build_nc(cfg):
    from concourse import bass, bacc, mybir, tile
    f32 = mybir.dt.float32
    f16 = mybir.dt.float16
    i32 = mybir.dt.int32
    AF = mybir.ActivationFunctionType
    OP = mybir.AluOpType

    nc = bacc.Bacc("TRN2", target_bir_lowering=False, debug=False,
                   num_devices=NC_, num_swdge_queues=4)

    L = cfg["layers"]
    xbs = nc.declare_dram_parameter("xbs", [N0 // NC_, P], f16, isOutput=False)
    xbd = nc.declare_dram_parameter("xbd", [N1 // NC_, P], f16, isOutput=False)
    wt = {}
    for name, shape in [("was1", [P, H]), ("wbig1", [P, 260]),
                        ("wp1", [P, 256]), ("wp2", [256, 264]),
                        ("brow2", [1, 264]), ("wbig2", [256, 260]),
                        ("brow2b", [1, 260]), ("wp3", [256, 196]),
                        ("brow3", [1, 196]), ("wbig3", [256, 51]),
                        ("brow3b", [1, 51])]:
        wt[name] = nc.declare_dram_parameter(name, shape, f16, isOutput=False)
    par = []
    iota_in = nc.declare_dram_parameter("iota_f", [P, P], f16, isOutput=False)
    for li, lc in enumerate(L):
        d = {}
        T_tot = lc["T_tot"]
        d["off"] = nc.declare_dram_parameter(f"off{li}", [P, T_tot], i32, isOutput=False)
        d["idx"] = nc.declare_dram_parameter(f"idx{li}", [P, T_tot * 8], mybir.dt.int16, isOutput=False)
        d["dloc"] = nc.declare_dram_parameter(f"dloc{li}", [P, T_tot], f16, isOutput=False)
        d["stT"] = nc.declare_dram_parameter(f"stT{li}", [P, T_tot * P], f16, isOutput=False)
        par.append(d)
    out_d = nc.declare_dram_parameter("out", [N3 // NC_, C_OUT], f32, isOutput=True)

    SP = bool(int(os.environ.get("K3_SP", "0")))

    def indirect_gather(out_ap, table, off_ap):
        return nc.gpsimd.indirect_dma_start(
            out=out_ap, out_offset=None, in_=table,
            in_offset=bass.IndirectOffsetOnAxis(ap=off_ap, axis=0))

    with tile.TileContext(nc) as tc:
        with (
            tc.tile_pool(name="const", bufs=1) as constp,
            tc.tile_pool(name="persist", bufs=1) as perp,
            tc.tile_pool(name="g", bufs=3) as gp,
            tc.tile_pool(name="st", bufs=3) as stp,
            tc.tile_pool(name="sT", bufs=3) as sTp,
            tc.tile_pool(name="sm", bufs=3) as smp,
            tc.tile_pool(name="mxp", bufs=2) as mxp,
            tc.tile_pool(name="fin", bufs=2) as finp,
            tc.tile_pool(name="pad", bufs=2, space="PSUM") as ppad,
            tc.tile_pool(name="pagg", bufs=2, space="PSUM") as pagg,
            tc.tile_pool(name="ptr", bufs=2, space="PSUM") as ptr,
            tc.tile_pool(name="pout", bufs=2, space="PSUM") as pout,
            tc.tile_pool(name="dram", bufs=1, space="DRAM") as dramp,
        ):
            from concourse.masks import make_identity
            ident = constp.tile([P, P], f16, tag="ident")
            make_identity(nc, ident[:])
            iota = constp.tile([P, P], f16, tag="iota")
            nc.sync.dma_start(out=iota[:], in_=iota_in[:, :])
            ones = constp.tile([1, P], f16, tag="ones")
            nc.vector.memset(ones[0:1, :], 1.0)
            ocol = constp.tile([P, 1], f32, tag="ocol")
            nc.vector.memset(ocol[:, 0:1], 1.0)
            mcol = constp.tile([P, 1], f32, tag="mcol")
            nc.vector.memset(mcol[:, 0:1], -1.0)

            wsb = {}
            for name, kch in [("was1", 1), ("wbig1", 1), ("wp1", 1),
                              ("wp2", 2), ("wbig2", 2), ("wp3", 2),
                              ("wbig3", 2)]:
                cols = wt[name].shape[1]
                wsb[name] = [constp.tile([P, cols], f16, tag=f"{name}_{k}",
                                         name=f"{name}_{k}")
                             for k in range(kch)]
                for k in range(kch):
                    nc.sync.dma_start(out=wsb[name][k][:],
                                      in_=wt[name][k * P:(k + 1) * P, :])
            for name in ["brow2", "brow2b", "brow3", "brow3b"]:
                cols = wt[name].shape[1]
                wsb[name] = constp.tile([1, cols], f16, tag=name, name=name)
                nc.sync.dma_start(out=wsb[name][0:1, :], in_=wt[name][0:1, :])

            lt = []
            for li, lc in enumerate(L):
                nch, Cs = lc["nch"], lc["Cs"]
                dd = {}
                dd["skip"] = perp.tile([P, nch * Cs], f16, tag=f"skip{li}",
                                       name=f"skip{li}")
                dd["adsb"] = perp.tile([P, nch * H], f16, tag=f"adsb{li}",
                                       name=f"adsb{li}")
                lt.append(dd)

            # gather tables (lo/hi halves written by piece AllGathers)
            QRT = (N0 // NC_) // 2          # 8192 rows per half per core
            xt_own = [dramp.tile([QRT, P], f16, tag=f"xt_own{k}",
                                 name=f"xt_own{k}") for k in range(2)]
            xt_half = [dramp.tile([QRT * NC_, P], f16, tag=f"xt_half{k}",
                                  name=f"xt_half{k}", addr_space="Shared")
                       for k in range(2)]
            h1_lo = dramp.tile([1024, 256], f16, tag="h1_lo")
            p1_own = [dramp.tile([2048, W2T], f16, tag=f"p1_own{k}",
                                 name=f"p1_own{k}") for k in range(2)]
            p1_half = [dramp.tile([2048 * NC_, W2T], f16, tag=f"p1_half{k}",
                                  name=f"p1_half{k}", addr_space="Shared")
                       for k in range(2)]
            h2_lo = dramp.tile([256, 256], f16, tag="h2_lo")
            p2_own = dramp.tile([1024, W3T], f16, tag="p2_own")
            p2_all = dramp.tile([1024 * NC_, W3T], f16, tag="p2_all",
                                addr_space="Shared")

            def transpose_to_sbuf(src_ap, tagbase):
                tp = ptr.tile([P, P], f16, tag="tp", space="PSUM")
                ncols = src_ap.shape[-1]
                nc.tensor.transpose(out=tp[0:ncols, :], in_=src_ap,
                                    identity=ident[:])
                ts = smp.tile([P, P], f16, tag=tagbase)
                nc.vector.tensor_copy(out=ts[0:ncols, :], in_=tp[0:ncols, :])
                return ts

            # ---------------- xt build
            XB = 8
            n_per_q = QRT // (P * XB)       # 8
            for k in range(2):
                for i in range(n_per_q):
                    r0 = k * QRT + i * XB * P
                    t0 = gp.tile([P, XB * P], f16, tag="xtb")
                    nc.sync.dma_start(
                        out=t0[:].rearrange("p (j f) -> p j f", f=P),
                        in_=xbs[r0:r0 + XB * P, :].rearrange(
                            "(j p) f -> p j f", p=P))
                    for j in range(XB):
                        ts = transpose_to_sbuf(t0[:, j * P:(j + 1) * P], "xtT")
                        pas = ppad.tile([P, H], f32, tag="pad", space="PSUM")
                        nc.tensor.matmul(out=pas[:, :], lhsT=ts[:],
                                         rhs=wsb["was1"][0][:], start=True, stop=True)
                        nc.vector.tensor_copy(out=t0[:, j * P + 101:j * P + 101 + H],
                                              in_=pas[:, :])
                    nc.sync.dma_start(
                        out=xt_own[k][i * XB * P:(i + 1) * XB * P, :].rearrange(
                            "(j p) f -> p j f", p=P),
                        in_=t0[:].rearrange("p (j f) -> p j f", f=P))
                nc.gpsimd.collective_compute(
                    "AllGather", mybir.AluOpType.bypass,
                    replica_groups=[list(range(NC_))],
                    ins=[xt_own[k][:].opt()],
                    outs=[xt_half[k][:].opt()])

            # ---------------- L1 phase A (local xbd)
            for i in range(L[0]["nch"]):
                t0 = gp.tile([P, P], f16, tag="pha1")
                nc.sync.dma_start(out=t0[:], in_=xbd[i * P:(i + 1) * P, :])
                ts = transpose_to_sbuf(t0[:], "pha1T")
                pa = pout.tile([P, 264], f32, tag="pa", space="PSUM")
                nc.tensor.matmul(out=pa[:, 0:260], lhsT=ts[:],
                                 rhs=wsb["wbig1"][0][:], start=True, stop=True)
                nc.vector.tensor_copy(out=lt[0]["adsb"][:, i * H:(i + 1) * H],
                                      in_=pa[:, 0:4])
                nc.vector.tensor_copy(out=lt[0]["skip"][:, i * 256:(i + 1) * 256],
                                      in_=pa[:, 4:260])

            # ---------------- phase A for L2/L3: local h rows
            def phase_a(li, h_lo, wbig, brow, Cs):
                lc = L[li]
                nch = lc["nch"]
                for i in range(nch):
                    hd = smp.tile([P, 256], f16, tag=f"ph{li}")
                    nc.sync.dma_start(out=hd[:], in_=h_lo[i * P:(i + 1) * P, :])
                    pa = pout.tile([P, 264], f32, tag="pa", space="PSUM")
                    for k in range(2):
                        ts = transpose_to_sbuf(hd[:, k * P:(k + 1) * P], "phT")
                        nc.tensor.matmul(out=pa[:, 0:4 + Cs], lhsT=ts[:],
                                         rhs=wbig[k][:], start=(k == 0), stop=False)
                    nc.tensor.matmul(out=pa[:, 0:4 + Cs], lhsT=ones[0:1, :],
                                     rhs=brow[0:1, :], start=False, stop=True)
                    nc.vector.tensor_copy(out=lt[li]["adsb"][:, i * H:(i + 1) * H],
                                          in_=pa[:, 0:4])
                    nc.vector.tensor_copy(out=lt[li]["skip"][:, i * Cs:(i + 1) * Cs],
                                          in_=pa[:, 4:4 + Cs])

            # ---------------- edge phase
            GCAP = 2048

            def edge_layer(li, gwins, itabs, mech, finalize):
                lc = L[li]
                nch, tpcb, ncpb = lc["nch"], lc["tpcb"], lc["ncpb"]
                E = lc["elem"]
                CM = lc["cm"]
                NA = CM * H
                aoff = lc["aoff"]
                T_MAX = max(sum(r) for r in tpcb)
                dd = lt[li]
                has_ind = any('I' in row for row in mech)
                t0c = 0
                for i in range(nch):
                    T = sum(tpcb[i])
                    if has_ind:
                        offs = smp.tile([P, T_MAX], i32, tag="offs")
                        nc.sync.dma_start(out=offs[:, :T],
                                          in_=par[li]["off"][:, t0c:t0c + T])
                    ixw = smp.tile([P, T_MAX * 8], mybir.dt.int16, tag="ixw")
                    nc.sync.dma_start(out=ixw[:, :T * 8],
                                      in_=par[li]["idx"][:, 8 * t0c:8 * (t0c + T)])
                    dl = smp.tile([P, T_MAX], f16, tag="dl")
                    nc.sync.dma_start(out=dl[:, :T],
                                      in_=par[li]["dloc"][:, t0c:t0c + T])
                    sT = sTp.tile([P, T_MAX * P], f16, tag="sT")
                    nc.sync.dma_start(out=sT[:, :T * P],
                                      in_=par[li]["stT"][:, P * t0c:P * (t0c + T)])
                    st = stp.tile([P, T_MAX * P], f16, tag="st")
                    nc.vector.tensor_tensor(
                        out=st[:, :T * P].rearrange("p (t d) -> p t d", d=P),
                        in0=dl[:, :T].rearrange("p (t o) -> p t o", o=1).to_broadcast([P, T, P]),
                        in1=iota[:, :].rearrange("p (o d) -> p o d", o=1).to_broadcast([P, T, P]),
                        op=OP.is_equal)
                    g0 = gp.tile([P, T_MAX * E], f16, tag="g0")
                    if i < 3:
                        nc.vector.memset(g0[:], 0.0)
                    off_t = 0
                    for b in range(len(tpcb[i])):
                        Tb = tpcb[i][b]
                        if Tb == 0:
                            continue
                        if mech[i][b] == 'I':
                            indirect_gather(
                                g0[:, off_t * E:(off_t + Tb) * E].rearrange(
                                    "p (j r) -> p j r", r=E),
                                itabs[b][:, :],
                                offs[:, off_t:off_t + Tb])
                            off_t += Tb
                        else:
                            ncall = ncpb[i][b]
                            s0 = 0
                            while ncall > 0:
                                n_this = min(ncall, GCAP)
                                tile_span = -(-n_this // P)
                                o2 = off_t + s0
                                nc.gpsimd.dma_gather(
                                    out_ap=g0[:, o2 * E:(o2 + tile_span) * E].rearrange(
                                        "p (j r) -> p j r", r=E),
                                    in_ap=gwins[b],
                                    idxs_ap=ixw[:, 8 * o2:8 * o2 + (-(-n_this // 16))],
                                    num_idxs=n_this, num_idxs_reg=n_this,
                                    elem_size=E, single_packet=SP)
                                ncall -= n_this
                                s0 += tile_span
                            off_t += Tb
                    GSZ = 6
                    padc = ppad.tile([P, T_MAX * H], f32, tag="pad", space="PSUM")
                    for t in range(T):
                        nc.tensor.matmul(out=padc[:, t * H:(t + 1) * H],
                                         lhsT=sT[:, t * P:(t + 1) * P],
                                         rhs=dd["adsb"][:, i * H:(i + 1) * H],
                                         start=True, stop=True)
                    eb = smp.tile([P, T_MAX * H], f32, tag="eb")
                    lk = smp.tile([P, T_MAX * H], f32, tag="lk")
                    exb = smp.tile([P, T_MAX * H], f16, tag="exb")
                    mxc = mxp.tile([P, T_MAX * NA], f16, tag="mx")
                    agg = pagg.tile([P, NA], f32, tag="agg", space="PSUM")
                    for gti in range(0, T, GSZ):
                        gn = min(GSZ, T - gti)
                        sl4 = slice(gti * H, (gti + gn) * H)
                        nc.vector.tensor_tensor(
                            out=eb[:, sl4].rearrange("p (t h) -> p t h", h=H),
                            in0=g0[:, gti * E:(gti + gn) * E].rearrange(
                                "p (t r) -> p t r", r=E)[:, :, aoff:aoff + H],
                            in1=padc[:, sl4].rearrange("p (t h) -> p t h", h=H),
                            op=OP.add)
                        nc.scalar.activation(out=lk[:, sl4], in_=eb[:, sl4],
                                             func=AF.Copy, scale=0.2)
                        nc.vector.tensor_tensor(out=eb[:, sl4], in0=eb[:, sl4],
                                                in1=lk[:, sl4], op=OP.max)
                        nc.scalar.activation(out=exb[:, sl4], in_=eb[:, sl4],
                                             func=AF.Exp)
                        for hh in range(H):
                            if li == 0:
                                in0 = g0[:, gti * E:(gti + gn) * E].rearrange(
                                    "p (t r) -> p t r", r=E)[:, :, 0:CM]
                            else:
                                in0 = g0[:, gti * E:(gti + gn) * E].rearrange(
                                    "p (t r) -> p t r", r=E)[:, :, hh * CM:(hh + 1) * CM]
                            nc.vector.tensor_tensor(
                                out=mxc[:, gti * NA:(gti + gn) * NA].rearrange(
                                    "p (t h c) -> p t h c", c=CM, h=H)[:, :, hh, :],
                                in0=in0,
                                in1=exb[:, sl4].rearrange(
                                    "p (t h) -> p t h", h=H)[:, :, hh:hh + 1].to_broadcast(
                                    [P, gn, CM]),
                                op=OP.mult)
                        for t in range(gti, gti + gn):
                            nc.tensor.matmul(out=agg[:, :],
                                             lhsT=st[:, t * P:(t + 1) * P],
                                             rhs=mxc[:, t * NA:(t + 1) * NA],
                                             start=(t == 0), stop=(t == T - 1))
                    finalize(i, agg)
                    t0c += T

            # ---------------- finalizers
            def proj_store(i, hc, wp, brow, pcols, owsel):
                pp = pout.tile([P, 264], f32, tag="pa", space="PSUM")
                for k in range(2):
                    ts = transpose_to_sbuf(hc[:, k * P:(k + 1) * P], "pjT")
                    nc.tensor.matmul(out=pp[:, 0:pcols], lhsT=ts[:],
                                     rhs=wp[k][:], start=(k == 0), stop=False)
                nc.tensor.matmul(out=pp[:, 0:pcols], lhsT=ones[0:1, :],
                                 rhs=brow[0:1, :], start=False, stop=True)
                pc = finp.tile([P, pcols], f16, tag="pc")
                nc.vector.tensor_copy(out=pc[:], in_=pp[:, 0:pcols])
                own, r0 = owsel(i)
                nc.sync.dma_start(out=own[r0:r0 + P, 0:pcols], in_=pc[:])

            def fin_concat(li, CM, hlo, nh_lo, owsel, wp, brow, pcols, post=None):
                dd = lt[li]

                def fin(i, agg):
                    ags = finp.tile([P, H * CM], f16, tag="ags")
                    nc.vector.tensor_copy(out=ags[:], in_=agg[:, :])
                    rc = finp.tile([P, H], f32, tag="rc")
                    nc.vector.reciprocal(
                        out=rc[:].rearrange("p (h o) -> p h o", o=1),
                        in_=ags[:].rearrange("p (h c) -> p h c", c=CM)[:, :, CM - 1:CM])
                    if li == 0:
                        po = pout.tile([P, 264], f32, tag="pa", space="PSUM")
                        for hh in range(H):
                            ts = transpose_to_sbuf(
                                ags[:, hh * CM:hh * CM + 100], "agT")
                            nc.tensor.matmul(
                                out=po[:, hh * 64:(hh + 1) * 64],
                                lhsT=ts[0:100, :],
                                rhs=wsb["wp1"][0][0:100, hh * 64:(hh + 1) * 64],
                                start=True, stop=True)
                        ho = finp.tile([P, 256], f32, tag="ho")
                        nc.vector.tensor_tensor(
                            out=ho[:].rearrange("p (h c) -> p h c", c=64),
                            in0=po[:, 0:256].rearrange("p (h c) -> p h c", c=64),
                            in1=rc[:].rearrange("p (h o) -> p h o", o=1).to_broadcast([P, H, 64]),
                            op=OP.mult)
                    else:
                        ho = finp.tile([P, 256], f32, tag="ho")
                        nc.vector.tensor_tensor(
                            out=ho[:].rearrange("p (h c) -> p h c", c=64),
                            in0=ags[:].rearrange("p (h c) -> p h c", c=CM)[:, :, 0:64],
                            in1=rc[:].rearrange("p (h o) -> p h o", o=1).to_broadcast([P, H, 64]),
                            op=OP.mult)
                    Cs = 256
                    nc.vector.tensor_tensor(out=ho[:, 0:Cs], in0=ho[:, 0:Cs],
                                            in1=dd["skip"][:, i * Cs:(i + 1) * Cs],
                                            op=OP.add)
                    mn = finp.tile([P, 256], f32, tag="mn")
                    nc.scalar.activation(out=mn[:, 0:Cs], in_=ho[:, 0:Cs], func=AF.Exp)
                    nc.vector.tensor_tensor(out=mn[:, 0:Cs], in0=mn[:, 0:Cs],
                                            in1=ocol[:, 0:1].to_broadcast([P, Cs]),
                                            op=OP.min)
                    nc.scalar.activation(out=ho[:, 0:Cs], in_=ho[:, 0:Cs], func=AF.Relu)
                    nc.vector.tensor_tensor(out=ho[:, 0:Cs], in0=ho[:, 0:Cs],
                                            in1=mn[:, 0:Cs], op=OP.add)
                    hc = finp.tile([P, 256], f16, tag="hc")
                    nc.vector.tensor_tensor(out=hc[:, 0:Cs], in0=ho[:, 0:Cs],
                                            in1=mcol[:, 0:1].to_broadcast([P, Cs]),
                                            op=OP.add)
                    if i < nh_lo:
                        nc.sync.dma_start(out=hlo[i * P:(i + 1) * P, :], in_=hc[:, 0:Cs])
                    proj_store(i, hc[:, 0:256], wp, brow, pcols, owsel)
                    if post is not None:
                        post(i)
                return fin

            def fin_l3(i, agg):
                CM, CH = 48, 47
                ags = finp.tile([P, H * CM], f16, tag="ags")
                nc.vector.tensor_copy(out=ags[:], in_=agg[:, :])
                rc = finp.tile([P, H], f32, tag="rc")
                nc.vector.reciprocal(
                    out=rc[:].rearrange("p (h o) -> p h o", o=1),
                    in_=ags[:].rearrange("p (h c) -> p h c", c=CM)[:, :, CH:CH + 1])
                hm = finp.tile([P, H * CH], f32, tag="hm")
                nc.vector.tensor_tensor(
                    out=hm[:].rearrange("p (h c) -> p h c", c=CH),
                    in0=ags[:].rearrange("p (h c) -> p h c", c=CM)[:, :, 0:CH],
                    in1=rc[:].rearrange("p (h o) -> p h o", o=1).to_broadcast([P, H, CH]),
                    op=OP.mult)
                ho = finp.tile([P, C_OUT], f32, tag="ho3")
                nc.vector.tensor_tensor(out=ho[:], in0=hm[:, 0:C_OUT],
                                        in1=hm[:, C_OUT:2 * C_OUT], op=OP.add)
                nc.vector.tensor_tensor(out=ho[:], in0=ho[:],
                                        in1=hm[:, 2 * C_OUT:3 * C_OUT], op=OP.add)
                nc.vector.tensor_tensor(out=ho[:], in0=ho[:],
                                        in1=hm[:, 3 * C_OUT:4 * C_OUT], op=OP.add)
                nc.vector.tensor_scalar(out=ho[:], in0=ho[:], scalar1=0.25,
                                        scalar2=None, op0=OP.mult)
                nc.vector.tensor_tensor(out=ho[:], in0=ho[:],
                                        in1=lt[2]["skip"][:, i * C_OUT:(i + 1) * C_OUT],
                                        op=OP.add)
                mx = finp.tile([P, 1], f32, tag="mx3")
                nc.vector.tensor_reduce(out=mx[:, 0:1], in_=ho[:],
                                        axis=mybir.AxisListType.X, op=OP.max)
                z = finp.tile([P, C_OUT], f32, tag="z3")
                nc.vector.tensor_tensor(
                    out=z[:], in0=ho[:],
                    in1=mx[:, 0:1].to_broadcast([P, C_OUT]), op=OP.subtract)
                ez = finp.tile([P, C_OUT], f32, tag="ez3")
                nc.scalar.activation(out=ez[:], in_=z[:], func=AF.Exp)
                sm = finp.tile([P, 1], f32, tag="sm3")
                nc.vector.tensor_reduce(out=sm[:, 0:1], in_=ez[:],
                                        axis=mybir.AxisListType.X, op=OP.add)
                ln = finp.tile([P, 1], f32, tag="ln3")
                nc.scalar.activation(out=ln[:, 0:1], in_=sm[:, 0:1], func=AF.Ln)
                zo = finp.tile([P, C_OUT], f32, tag="zo3")
                nc.vector.tensor_tensor(
                    out=zo[:], in0=z[:],
                    in1=ln[:, 0:1].to_broadcast([P, C_OUT]), op=OP.subtract)
                nc.sync.dma_start(out=out_d[i * P:(i + 1) * P, :], in_=zo[:])

            # ---------------- run layers
            def post1(i):
                if i % 16 == 15:
                    k = i // 16
                    nc.gpsimd.collective_compute(
                        "AllGather", mybir.AluOpType.bypass,
                        replica_groups=[list(range(NC_))],
                        ins=[p1_own[k][:].opt()],
                        outs=[p1_half[k][:].opt()])

            def post2(i):
                if i == 7:
                    nc.gpsimd.collective_compute(
                        "AllGather", mybir.AluOpType.bypass,
                        replica_groups=[list(range(NC_))],
                        ins=[p2_own[:].opt()],
                        outs=[p2_all[:].opt()])

            fin1 = fin_concat(0, 101, h1_lo, 8,
                              lambda i: (p1_own[i // 16], (i % 16) * P),
                              wsb["wp2"], wsb["brow2"], 264, post=post1)
            gwins1 = [xt_half[0][0:32768, :], xt_half[0][32768:65536, :],
                      xt_half[1][0:32768, :], xt_half[1][32768:65536, :]]
            itabs1 = [xt_half[0][:, :], xt_half[0][:, :],
                      xt_half[1][:, :], xt_half[1][:, :]]
            edge_layer(0, gwins1, itabs1, cfg["mech"][0], fin1)

            phase_a(1, h1_lo, wsb["wbig2"], wsb["brow2b"], 256)
            fin2 = fin_concat(1, 65, h2_lo, 2,
                              lambda i: (p2_own, i * P),
                              wsb["wp3"], wsb["brow3"], 196, post=post2)
            gwins2 = [p1_half[0][:, :], p1_half[1][:, :]]
            edge_layer(1, gwins2, [None, None], cfg["mech"][1], fin2)

            phase_a(2, h2_lo, wsb["wbig3"], wsb["brow3b"], 47)
            edge_layer(2, [p2_all[:, :]], [None], cfg["mech"][2], fin_l3)
    nc.compile()
    return nc


# ---------------------------------------------------------------- entry

def prepare(inputs):
    x = np.asarray(inputs["x"], np.float32)
    sch1, tpcb1, ncpb1, nch1 = _build_sched(
        inputs["src1"], inputs["dst1"], OWN1, LOC1, _t1_bucket_row, N1 // NC_, 4)
    sch2, tpcb2, ncpb2, nch2 = _build_sched(
        inputs["src2"], inputs["dst2"], OWN2, LOC2, _t2_bucket_row, N2 // NC_, 2)
    sch3, tpcb3, ncpb3, nch3 = _build_sched(
        inputs["src3"], inputs["dst3"], OWN3, LOC3, _t3_bucket_row, N3 // NC_, 1)
    w = _prep_weights({k: np.asarray(v, np.float32) if v.dtype != np.int32 else v
                       for k, v in inputs.items()})

    IND = bool(int(os.environ.get("K3_IND", "0")))
    mech1 = [['I' if (IND and b == (i % 4)) else 'G' for b in range(4)]
             for i in range(nch1)]
    mech2 = [['G', 'G'] for _ in range(nch2)]
    mech3 = [['G'] for _ in range(nch3)]
    cfg = {"layers": [
        dict(T_tot=sum(sum(r) for r in tpcb1), nch=nch1, tpcb=tpcb1,
             ncpb=ncpb1, elem=W1T, cm=101, aoff=101, Cs=256),
        dict(T_tot=sum(sum(r) for r in tpcb2), nch=nch2, tpcb=tpcb2,
             ncpb=ncpb2, elem=W2T, cm=65, aoff=260, Cs=256),
        dict(T_tot=sum(sum(r) for r in tpcb3), nch=nch3, tpcb=tpcb3,
             ncpb=ncpb3, elem=W3T, cm=48, aoff=192, Cs=47),
    ], "mech": [mech1, mech2, mech3]}
    nc = _build_nc(cfg)

    xb = np.zeros((N0, P), np.float16)
    xb[:, :F_IN] = x.astype(np.float16)
    xb[:, F_IN] = 1.0
    iota_f = np.tile(np.arange(P, dtype=np.float16)[None, :], (P, 1))

    # local dst x rows per core (tier order)
    s1rows = []
    for c in range(NC_):
        rows = [np.arange(s + c * wd, s + (c + 1) * wd) for s, wd in TIERS1]
        s1rows.append(np.concatenate(rows))

    in_maps = []
    for c in range(NC_):
        m = dict(w)
        m["xbs"] = np.ascontiguousarray(xb[c * (N0 // NC_):(c + 1) * (N0 // NC_)])
        m["xbd"] = np.ascontiguousarray(xb[s1rows[c]])
        m["iota_f"] = iota_f
        for li, sch in enumerate([sch1, sch2, sch3]):
            s = sch[c]
            m[f"off{li}"] = s["off"]
            m[f"idx{li}"] = s["idxw"]
            m[f"dloc{li}"] = s["dloc"]
            m[f"stT{li}"] = s["stT"]
        in_maps.append(m)
    return nc, in_maps


def kernel(**inputs):
    from concourse import bass_utils
    nc, in_maps = prepare(inputs)
    res = bass_utils.run_bass_kernel_spmd(nc, in_maps, list(range(NC_)),
                                          trace=False)
    out = np.zeros((N3, C_OUT), np.float32)
    for c in range(NC_):
        out[c * 256:(c + 1) * 256] = res.results[c]["out"]
    return out
